# revision 36
# baseline (speedup 1.0000x reference)
# GATConv kernel for Trainium2 (Bass/Tile), 8-core data parallel over batch.
#
# Problem (hardcoded from nn_GATConv_54692113547387):
#   x   [8, 1024, 128] f32, adj [8, 1024, 1024] i32,
#   W   [128, 128] f32,  b [128] f32,  a [64] f32
#   out [8, 1024, 128] f32
#   h = x @ W.T + b, viewed [N, H=4, D=32]
#   e[h,i,j] = leaky_relu(s[h,i] + t[h,j], 0.2); masked where adj==0
#   attn = softmax_j(e);  out[i,(h,d)] = sum_j attn[h,i,j] h[j,h,d]
#
# Math used here (exact reformulation):
#   exp(lrelu(u)) = max(exp(u), exp(0.2 u)) for u = s_i + t_j.  Dividing row i
#   by exp(0.2 s_i) (cancels in softmax):
#     P[j,i] = adj[i,j] * max(sE_i * tE_j, D_j)
#   with sE = exp(0.8 s), tE = exp(t), D = exp(0.2 t) -- all N-sized vectors,
#   so NO elementwise transcendental is needed on the N x N attention matrix:
#   one DVE dual-op tensor_scalar (mult, max) builds it at 4x rate.
#   out_unnorm^T[(h,d)|sum, i] = sum_j [H_h | 1][j,:]^T P[j,i]  (PE matmul,
#   stationary [33] incl. a ones column that yields the softmax denominator),
#   then out[i,hd] = U[d,i]/U[32,i].
import numpy as np

import concourse.mybir as mybir
import concourse.tile as tile
from concourse import bacc
from concourse.masks import make_identity

F32 = mybir.dt.float32
F16 = mybir.dt.float16
I32 = mybir.dt.int32

P = 128          # partitions
N = 1024         # nodes
NT = N // P      # 8 node tiles
H = 4            # heads
D = 32           # head dim
DE = D + 1       # head dim + rowsum column
NCORES = 8

# Tuning knobs (module-level so experiments can override before build).
GP_JT_MIN = 7      # jt >= this runs its mask multiply on GPSIMD
W_BUFS = 6         # z/p tile double-buffering depth
OUT_MODE = "pair"  # "head": per-head output phase; "pair": two heads batched


# (h, jt) pairs whose mask multiply runs on GPSIMD instead of DVE.
def _on_gpsimd(h, jt):
    return jt >= GP_JT_MIN


def build_nc(use_gpsimd=True, repeat=1):
    nc = bacc.Bacc("TRN2", target_bir_lowering=False, debug=False)

    x_d = nc.dram_tensor("x", [N, P], F32, kind="ExternalInput")
    adj_d = nc.dram_tensor("adj", [N, N], I32, kind="ExternalInput")
    w_d = nc.dram_tensor("W", [P, P], F32, kind="ExternalInput")
    b_d = nc.dram_tensor("b", [P], F32, kind="ExternalInput")
    a_d = nc.dram_tensor("a", [2 * D], F32, kind="ExternalInput")
    out_d = nc.dram_tensor("out", [N, P], F32, kind="ExternalOutput")

    x_view = x_d[:].rearrange("(t p) i -> p t i", p=P)      # [128, 8, 128]
    adj_view = adj_d[:].rearrange("(t p) j -> p t j", p=P)  # [128, 8, 1024]
    out_view = out_d[:].rearrange("(t p) o -> p t o", p=P)  # [128, 8, 128]

    with tile.TileContext(nc) as tc:
        with (
            tc.tile_pool(name="const", bufs=1) as cpool,
            tc.tile_pool(name="work", bufs=W_BUFS) as wpool,
            tc.tile_pool(name="outp", bufs=3) as opool,
            tc.tile_pool(name="dram", bufs=1, space="DRAM") as dpool,
            tc.tile_pool(name="psmisc", bufs=2, space="PSUM") as psmisc,
            tc.tile_pool(name="psagg", bufs=4, space="PSUM") as psagg,
            tc.tile_pool(name="psout", bufs=2, space="PSUM") as psout,
        ):
            # ---------------- constants / inputs ----------------
            ident = cpool.tile([P, P], F32, tag="ident")
            make_identity(nc, ident[:])

            x_sb = cpool.tile([P, NT, P], F32, tag="x")
            nc.sync.dma_start(x_sb[:], x_view)

            w_sb = cpool.tile([P, P], F32, tag="w")
            nc.sync.dma_start(w_sb[:], w_d[:])

            bias_col = cpool.tile([P, 1], F32, tag="bias")
            nc.sync.dma_start(bias_col[:], b_d[:, None])

            # ab_bd[o, c]: c in 0..3 -> a_src per head, 4..7 -> a_dst per head
            ab_bd = cpool.tile([P, 2 * H], F32, tag="ab")
            nc.vector.memset(ab_bd[:], 0.0)
            for h in range(H):
                nc.sync.dma_start(ab_bd[h * D:(h + 1) * D, h:h + 1],
                                  a_d[0:D, None])
                nc.sync.dma_start(
                    ab_bd[h * D:(h + 1) * D, H + h:H + h + 1], a_d[D:2 * D, None]
                )

            # persistent tiles (single-buffered; repeats serialize on them)
            adj_f = cpool.tile([P, NT, N], F16, tag="adjf")
            wt_sb = cpool.tile([P, P], F32, tag="wt")
            xt_sb = cpool.tile([P, N], F32, tag="xt")
            ht_sb = cpool.tile([P, N], F32, tag="ht")
            s16 = cpool.tile([H, N], F16, tag="s16")
            s16e = cpool.tile([H, N], F16, tag="s16e")
            t_sb = cpool.tile([H, N], F32, tag="t")
            sbc = cpool.tile([P, H, N], F16, tag="sbc")
            s_dram = dpool.tile([H, N], F16)
            tcols = cpool.tile([P, NT, H], F32, tag="tcols")
            dcols = cpool.tile([P, NT, H], F32, tag="dcols")
            ecols = cpool.tile([P, NT, H], F32, tag="ecols")
            hext = cpool.tile([P, NT, H * DE], F16, tag="hext")
            adjt = cpool.tile([P, NT, N], F16, tag="adjt")
            outT = [
                cpool.tile([DE, N], F32, tag=f"outT{h}", name=f"outT{h}")
                for h in range(H)
            ]

            for rep in range(repeat):

                # ---------------- features ----------------
                # WT[i, o] = W[o, i]
                ps = psmisc.tile([P, 512], F32, tag="m")
                nc.tensor.transpose(ps[:, 0:P], w_sb[:], ident[:])
                nc.vector.tensor_copy(wt_sb[:], ps[:, 0:P])

                # xT[i, n] = x[n, i] (4 transposes per PSUM bank, 1 copy)
                for g in range(2):
                    ps = psmisc.tile([P, 512], F32, tag="m")
                    for k in range(4):
                        t = g * 4 + k
                        nc.tensor.transpose(ps[:, k * P:(k + 1) * P],
                                            x_sb[:, t, :], ident[:])
                    nc.vector.tensor_copy(
                        xt_sb[:, g * 512:(g + 1) * 512], ps[:]
                    )

                # hT[o, n] = sum_i WT[i, o] xT[i, n] + b[o]
                for half in range(2):
                    sl = slice(half * 512, (half + 1) * 512)
                    ps = psmisc.tile([P, 512], F32, tag="m")
                    nc.tensor.matmul(ps[:], wt_sb[:], xt_sb[:, sl],
                                     start=True, stop=True)
                    nc.scalar.add(ht_sb[:, sl], ps[:], bias_col[:])

                # s[h, n], t[h, n]
                for half in range(2):
                    sl = slice(half * 512, (half + 1) * 512)
                    ps = psmisc.tile([P, 512], F32, tag="m")
                    nc.tensor.matmul(ps[0:H, :], ab_bd[:, 0:H], ht_sb[:, sl],
                                     start=True, stop=True)
                    nc.scalar.copy(s16[:, sl], ps[0:H, :])
                    ps2 = psmisc.tile([P, 512], F32, tag="m")
                    nc.tensor.matmul(ps2[0:H, :], ab_bd[:, H:2 * H], ht_sb[:, sl],
                                     start=True, stop=True)
                    nc.scalar.copy(t_sb[:, sl], ps2[0:H, :])

                # sE = exp(0.8 s) (tiny), then broadcast to all partitions
                nc.scalar.activation(
                    s16e[:], s16[:], mybir.ActivationFunctionType.Exp, scale=0.8
                )
                nc.sync.dma_start(s_dram[:], s16e[:])
                for h in range(H):
                    nc.sync.dma_start(
                        sbc[:, h, :], s_dram[h:h + 1, :].to_broadcast([P, N])
                    )

                # tcols[j_p, jt, h] = t[h, jt*128 + j_p] (per-partition ACT bias)
                for g in range(2):
                    ps = psmisc.tile([P, 512], F32, tag="m")
                    for k in range(4):
                        t = g * 4 + k
                        nc.tensor.transpose(
                            ps[:, k * H:(k + 1) * H],
                            t_sb[:, t * P:(t + 1) * P], ident[0:H, 0:H]
                        )
                    nc.scalar.copy(tcols[:, g * 4:(g + 1) * 4, :],
                                   ps[:, 0:4 * H].rearrange(
                                       "p (t h) -> p t h", h=H))

                # D_j = exp(0.2 t_j), tE_j = exp(t_j)
                nc.scalar.activation(
                    dcols[:], tcols[:], mybir.ActivationFunctionType.Exp, scale=0.2
                )
                nc.scalar.activation(
                    ecols[:], tcols[:], mybir.ActivationFunctionType.Exp
                )

                # h natural + ones column: hext[n_p, jt, h*33 + (0..31 | 32)]
                for g in range(2):
                    ps = psmisc.tile([P, 512], F32, tag="m")
                    for k in range(4):
                        t = g * 4 + k
                        nc.tensor.transpose(ps[:, k * P:(k + 1) * P],
                                            ht_sb[:, t * P:(t + 1) * P], ident[:])
                    dst = (hext[:, g * 4:(g + 1) * 4, :]
                           .rearrange("p t (h e) -> p t h e", h=H)[:, :, :, 0:D])
                    srcap = ps[:].rearrange("p (t h e) -> p t h e", t=4, h=H)
                    nc.scalar.copy(dst, srcap)
                ones_ap = hext[:].rearrange("p t (h e) -> p t h e", h=H)[:, :, :, D]
                nc.vector.memset(ones_ap, 1.0)


                # ------- adjacency: SWDGE cast-load (int32->f16) + xbar
                # transpose, one i-row-block at a time on both HWDGE rings.
                for it in range(NT):
                    nc.gpsimd.dma_start(adj_f[:, it, :], adj_view[:, it, :])
                    nc.sync.dma_start_transpose(
                        adjt[:, :, it * P:(it + 1) * P], adj_f[:, it, :]
                    )
                # ---------------- main loop ----------------
                out_sb = cpool.tile([P, NT, P], F32, tag="outsb")
                for h in range(H):
                    acc = [
                        psagg.tile([DE, 512], F32, tag="agg",
                                   name=f"acc{rep}_{h}_{i}")
                        for i in range(2)
                    ]
                    for jt in range(NT):
                        # z = max(sE_i * tE_j, D_j) in one 4x-mode dual-op
                        z = wpool.tile([P, N], F16, tag="z")
                        nc.vector.tensor_scalar(
                            z[:], sbc[:, h, :],
                            ecols[:, jt, h:h + 1], dcols[:, jt, h:h + 1],
                            mybir.AluOpType.mult, mybir.AluOpType.max,
                        )
                        p_t = wpool.tile([P, N], F16, tag="p")
                        eng = (
                            nc.gpsimd
                            if use_gpsimd and _on_gpsimd(h, jt)
                            else nc.vector
                        )
                        for ih in range(2):
                            sl2 = slice(ih * 512, (ih + 1) * 512)
                            eng.tensor_tensor(
                                p_t[:, sl2], z[:, sl2], adjt[:, jt, sl2],
                                mybir.AluOpType.mult,
                            )
                            nc.tensor.matmul(
                                acc[ih][:],
                                hext[:, jt, h * DE:(h + 1) * DE],
                                p_t[:, sl2],
                                start=(jt == 0), stop=(jt == NT - 1),
                            )
                    for ih in range(2):
                        nc.scalar.copy(
                            outT[h][:, ih * 512:(ih + 1) * 512], acc[ih][:]
                        )
                    if OUT_MODE == "head":
                        # output phase for this head: transpose back + normalize
                        for it in range(NT):
                            po = psout.tile([P, DE], F32, tag="po")
                            sl = slice(it * P, (it + 1) * P)
                            nc.tensor.transpose(
                                po[:], outT[h][:, sl], ident[0:DE, 0:DE]
                            )
                            r = opool.tile([P, 1], F32, tag="r")
                            nc.vector.reciprocal(r[:], po[:, D:DE])
                            nc.vector.tensor_tensor(
                                out_sb[:, it, h * D:(h + 1) * D]
                                [:, None, :].rearrange("p u e -> p (u e)"),
                                po[:, 0:D],
                                r[:].to_broadcast([P, D]),
                                mybir.AluOpType.mult,
                            )
                    elif OUT_MODE == "pair" and h % 2 == 1:
                        # paired output phase after heads (h-1, h)
                        for it in range(NT):
                            po = psout.tile([P, 2 * DE], F32, tag="po")
                            sl = slice(it * P, (it + 1) * P)
                            nc.tensor.transpose(
                                po[:, 0:DE], outT[h - 1][:, sl], ident[0:DE, 0:DE]
                            )
                            nc.tensor.transpose(
                                po[:, DE:2 * DE], outT[h][:, sl], ident[0:DE, 0:DE]
                            )
                            po3 = po[:].rearrange("p (u e) -> p u e", u=2)
                            r = opool.tile([P, 2], F32, tag="r")
                            nc.vector.reciprocal(r[:], po3[:, :, D])
                            nc.vector.tensor_tensor(
                                out_sb[:, it, (h - 1) * D:(h + 1) * D]
                                .rearrange("p (u e) -> p u e", u=2),
                                po3[:, :, 0:D],
                                r[:, :, None].to_broadcast([P, 2, D]),
                                mybir.AluOpType.mult,
                            )

                if OUT_MODE == "tail":
                    for pair in range(2):
                        for it in range(NT):
                            po = psout.tile([P, 2 * DE], F32, tag="po")
                            sl = slice(it * P, (it + 1) * P)
                            nc.tensor.transpose(
                                po[:, 0:DE], outT[2 * pair][:, sl],
                                ident[0:DE, 0:DE]
                            )
                            nc.tensor.transpose(
                                po[:, DE:2 * DE], outT[2 * pair + 1][:, sl],
                                ident[0:DE, 0:DE]
                            )
                            po3 = po[:].rearrange("p (u e) -> p u e", u=2)
                            r = opool.tile([P, 2], F32, tag="r")
                            nc.vector.reciprocal(r[:], po3[:, :, D])
                            nc.vector.tensor_tensor(
                                out_sb[:, it, 2 * pair * D:(2 * pair + 2) * D]
                                .rearrange("p (u e) -> p u e", u=2),
                                po3[:, :, 0:D],
                                r[:, :, None].to_broadcast([P, 2, D]),
                                mybir.AluOpType.mult,
                            )
                nc.scalar.dma_start(out_view[:], out_sb[:])
                if OUT_MODE == "tail":
                    for pair in range(2):
                        for it in range(NT):
                            po = psout.tile([P, 2 * DE], F32, tag="po")
                            sl = slice(it * P, (it + 1) * P)
                            nc.tensor.transpose(
                                po[:, 0:DE], outT[2 * pair][:, sl],
                                ident[0:DE, 0:DE]
                            )
                            nc.tensor.transpose(
                                po[:, DE:2 * DE], outT[2 * pair + 1][:, sl],
                                ident[0:DE, 0:DE]
                            )
                            po3 = po[:].rearrange("p (u e) -> p u e", u=2)
                            r = opool.tile([P, 2], F32, tag="r")
                            nc.vector.reciprocal(r[:], po3[:, :, D])
                            nc.vector.tensor_tensor(
                                out_sb[:, it, 2 * pair * D:(2 * pair + 2) * D]
                                .rearrange("p (u e) -> p u e", u=2),
                                po3[:, :, 0:D],
                                r[:, :, None].to_broadcast([P, 2, D]),
                                mybir.AluOpType.mult,
                            )
                nc.scalar.dma_start(out_view[:], out_sb[:])

    nc.compile()
    return nc


_NC_CACHE = {}

# Test-harness knobs (not used by the grading path).
TRACE = False
LAST_RESULT = None


def _get_nc():
    if "nc" not in _NC_CACHE:
        _NC_CACHE["nc"] = build_nc()
    return _NC_CACHE["nc"]


def kernel(x, adj, W, b, a):
    global LAST_RESULT
    from concourse.bass_utils import run_bass_kernel_spmd

    nc = _get_nc()
    x = np.asarray(x, dtype=np.float32)
    adj = np.asarray(adj, dtype=np.int32)
    W = np.ascontiguousarray(np.asarray(W, dtype=np.float32))
    b = np.ascontiguousarray(np.asarray(b, dtype=np.float32))
    a = np.ascontiguousarray(np.asarray(a, dtype=np.float32))

    in_maps = [
        {
            "x": np.ascontiguousarray(x[c]),
            "adj": np.ascontiguousarray(adj[c]),
            "W": W,
            "b": b,
            "a": a,
        }
        for c in range(NCORES)
    ]
    res = run_bass_kernel_spmd(
        nc, in_maps, core_ids=list(range(NCORES)), trace=TRACE
    )
    LAST_RESULT = res
    out = np.stack([res.results[c]["out"] for c in range(NCORES)], axis=0)
    return out.astype(np.float32)


if __name__ == "__main__":
    nc = build_nc()
    print("built OK")
